# revision 18
# baseline (speedup 1.0000x reference)
"""Trainium2 Bass kernel for nn_CausalSelfAttention_43009802502282.

Causal self-attention with external memory (mem_k == mem_v), B=4, T=2048,
C=1024, 16 heads x 64, MEM=256.

Sharding (8 cores): core c -> batch b = c//2, head-group g = c%2 (8 heads).
Each core computes qkv for its heads (column-sliced W_attn), attention for
its 8 heads, and a partial projection (row-sliced W_proj). Host sums the
two partials per batch and adds b_proj.

On-chip layout is "transposed" (k-major): scoresT[s, t] tiles are produced
by PE with K=64 contraction, exp on ScalarE (PSUM->SBUF, scale=1/8 folded
in, causal mask pre-added in PSUM via identity-matmul of an additive -1e30
mask), then AV with K=128 using v-with-ones-column (M=65) so the softmax
denominator falls out of the same matmul. All matmul inputs are float32r
(full PE rate at N>=512, ~1e-4 relative error).
"""
import sys

sys.path.insert(0, "/opt/trn_rl_repo")

import numpy as np
import concourse.bass as bass
import concourse.mybir as mybir
from concourse import bacc
from concourse.tile import TileContext
from concourse.bass_utils import run_bass_kernel_spmd

F32 = mybir.dt.float32
F32R = mybir.dt.float32r
EXP = mybir.ActivationFunctionType.Exp

B, T, C = 4, 2048, 1024
NH, D, MEM = 16, 64, 128 * 2  # MEM=256
HL = 8                        # heads per core
P = 128
S = MEM + T                   # 2304
NST = S // P                  # 18 s-tiles (0,1 = memory; 2..17 causal)
NCH = T // 512                # 4 chunks of 512
NKT = C // P                  # 8 contraction tiles
NEG = -1.0e30
SCALE = 0.125                 # 1/sqrt(64)

_CACHE: dict = {}


def _build(num_devices=8, reps=1, parts="full"):
    nc = bacc.Bacc("TRN2", target_bir_lowering=False, debug=False, num_devices=num_devices)

    xT_d = nc.dram_tensor("xT", [C, T], F32, kind="ExternalInput")
    memT_d = nc.dram_tensor("memT", [HL * D, MEM], F32, kind="ExternalInput")
    memn_d = nc.dram_tensor("memn", [MEM, HL * D], F32, kind="ExternalInput")
    wqk_d = nc.dram_tensor("wqk", [C, 1024], F32, kind="ExternalInput")
    wv_d = nc.dram_tensor("wv", [C, 512], F32, kind="ExternalInput")
    wp_d = nc.dram_tensor("wp", [512, C], F32, kind="ExternalInput")
    bqk_d = nc.dram_tensor("bqk", [1, 1024], F32, kind="ExternalInput")
    bv_d = nc.dram_tensor("bv", [1, 512], F32, kind="ExternalInput")
    masks_d = nc.dram_tensor("masks", [4, P, 512], F32, kind="ExternalInput")
    ident_d = nc.dram_tensor("ident", [P, P], F32, kind="ExternalInput")
    onesrow_d = nc.dram_tensor("onesrow", [1, 512], F32, kind="ExternalInput")
    ones2_d = nc.dram_tensor("ones2", [D + 1, D], F32, kind="ExternalInput")
    onescol_d = nc.dram_tensor("onescol", [P, NST * HL], F32, kind="ExternalInput")
    out_d = nc.dram_tensor("out", [T, C], F32, kind="ExternalOutput")

    import contextlib

    with TileContext(nc) as tc:
        with (tc.For_i(0, reps, 1) if reps > 1 else contextlib.nullcontext()):
            _body(nc, tc, dict(locals()), parts)
    nc.compile()
    return nc


def _body(nc, tc, dr, parts="full"):
    xT_d, memT_d, memn_d = dr["xT_d"], dr["memT_d"], dr["memn_d"]
    wqk_d, wv_d, wp_d = dr["wqk_d"], dr["wv_d"], dr["wp_d"]
    bqk_d, bv_d, masks_d = dr["bqk_d"], dr["bv_d"], dr["masks_d"]
    ident_d, onesrow_d, ones2_d = dr["ident_d"], dr["onesrow_d"], dr["ones2_d"]
    onescol_d, out_d = dr["onescol_d"], dr["out_d"]
    LAG = 2

    with tc.tile_pool(name="pers", bufs=1) as pers:
        # persistent tiles
        qkT = pers.tile([P, 8, T], F32R, tag="qkT")       # rows: q cols (tiles 0-3), k cols (4-7)
        v_s = pers.tile([P, NST, HL, D + 1], F32R, tag="v")
        memT_s = pers.tile([P, 4, MEM], F32R, tag="memT")
        masks_s = pers.tile([P, 4, 512], F32R, tag="masks")
        ident_s = pers.tile([P, P], F32R, tag="ident")
        bqk_s = pers.tile([1, 1024], F32R, tag="bqk")
        bv_s = pers.tile([1, 512], F32R, tag="bv")
        onesrow_s = pers.tile([1, 512], F32R, tag="onesrow")
        wv_s = pers.tile([P, NKT, 512], F32R, tag="wv")
        for c in range(NKT):
            eng = nc.sync if c % 2 == 0 else nc.gpsimd
            eng.dma_start(wv_s[:, c], wv_d[128 * c:128 * c + 128, :].bitcast(F32R))

        nc.sync.dma_start(memT_s[:], memT_d[:, :].rearrange("(rt p) m -> p rt m", p=P).bitcast(F32R))
        for st in range(2):
            nc.sync.dma_start(v_s[:, st, :, 0:D],
                              memn_d[128 * st:128 * st + 128, :]
                              .rearrange("p (j d) -> p j d", d=D).bitcast(F32R))
        nc.sync.dma_start(v_s[:, :, :, D],
                          onescol_d[:, :].rearrange("p (st j) -> p st j", j=HL).bitcast(F32R))
        nc.sync.dma_start(masks_s[:], masks_d[:, :, :].rearrange("m p t -> p m t").bitcast(F32R))
        nc.sync.dma_start(ident_s[:], ident_d[:, :].bitcast(F32R))
        nc.sync.dma_start(bqk_s[:], bqk_d[:, :].bitcast(F32R))
        nc.sync.dma_start(bv_s[:], bv_d[:, :].bitcast(F32R))
        nc.sync.dma_start(onesrow_s[:], onesrow_d[:, :].bitcast(F32R))

        for half in range(2):
            # ---- qkv projections for this T-half ----
            with tc.tile_pool(name=f"w1_{half}", bufs=1) as w1, \
                 tc.tile_pool(name=f"ps1_{half}", bufs=4, space="PSUM") as ps1:
                wqk_s = w1.tile([P, NKT, 1024], F32R, tag="wqk", name=f"wqk_{half}")
                xTh = w1.tile([P, NKT, 1024], F32R, tag="xTh", name=f"xTh_{half}")
                for c in range(NKT):
                    eng = nc.sync if c % 2 == 0 else nc.gpsimd
                    eng2 = nc.gpsimd if c % 2 == 0 else nc.sync
                    eng.dma_start(wqk_s[:, c], wqk_d[128 * c:128 * c + 128, :].bitcast(F32R))
                    eng2.dma_start(xTh[:, c], xT_d[128 * c:128 * c + 128,
                                                   1024 * half:1024 * half + 1024].bitcast(F32R))
                for nn in range(2):
                    ch = 2 * half + nn
                    for mt in range(8):
                        ps = ps1.tile([P, 512], F32, tag="qk", name=f"qk_{ch}_{mt}")
                        for c in range(NKT):
                            nc.tensor.matmul(ps[:], wqk_s[:, c, 128 * mt:128 * mt + 128],
                                             xTh[:, c, 512 * nn:512 * nn + 512],
                                             start=(c == 0), stop=False)
                        nc.tensor.matmul(ps[:], bqk_s[:, 128 * mt:128 * mt + 128], onesrow_s[:],
                                         start=False, stop=True)
                        nc.vector.tensor_copy(qkT[:, mt, 512 * ch:512 * ch + 512], ps[:])
                for tl in range(8):
                    tt = 8 * half + tl
                    ps = ps1.tile([P, 512], F32, tag="v", name=f"v_{tt}")
                    for c in range(NKT):
                        nc.tensor.matmul(ps[:], xTh[:, c, 128 * tl:128 * tl + 128], wv_s[:, c],
                                         start=(c == 0), stop=False)
                    nc.tensor.matmul(ps[:], onesrow_s[:, 0:128], bv_s[:], start=False, stop=True)
                    nc.vector.tensor_copy(v_s[:, 2 + tt, :, 0:D], ps[:])

            if parts == "p1":
                continue

            # ---- attention + projection for chunks 2*half, 2*half+1 ----
            # Head pairs (2r, 2r+1) sit at base partitions 0/64 of one
            # row-tile; interleaving their QK matmuls packs the PE row
            # groups (~183 ns/mm vs ~459 at a fixed base). One sc tile
            # [128, 1024] holds the pair scores for one s-tile -> a single
            # exp per strip. AV lags LAG strips behind QK so the in-order
            # PE queue never blocks on ScalarE. Projection contracts K=128
            # over pair-stacked y tiles (odd head restacked to partitions
            # 64..127 via SBUF-to-SBUF DMA).
            with tc.tile_pool(name=f"wp2_{half}", bufs=1) as wp2, \
                 tc.tile_pool(name=f"stgp_{half}", bufs=6) as stgp, \
                 tc.tile_pool(name=f"stkp_{half}", bufs=5) as stkp, \
                 tc.tile_pool(name=f"expp_{half}", bufs=4) as expp, \
                 tc.tile_pool(name=f"bcsp_{half}", bufs=1) as bcsp, \
                 tc.tile_pool(name=f"outp_{half}", bufs=3) as outp, \
                 tc.tile_pool(name=f"scps_{half}", bufs=3, space="PSUM") as scps, \
                 tc.tile_pool(name=f"avps_{half}", bufs=2, space="PSUM") as avps:

                wp2_s = wp2.tile([P, 4, C], F32R, tag="wp", name=f"wp_{half}")
                nc.gpsimd.dma_start(
                    wp2_s[:],
                    wp_d[:, :].rearrange("(rt p) c -> p rt c", p=P).bitcast(F32R))

                for nn in range(2):
                    ch = 2 * half + nn
                    stks = []
                    n_st = 6 + 4 * ch
                    for pr in range(4):
                        j0, j1 = 2 * pr, 2 * pr + 1
                        rt = pr
                        qT0 = qkT[0:64, rt, 512 * ch:512 * ch + 512]
                        qT1 = qkT[64:128, rt, 512 * ch:512 * ch + 512]
                        av0 = avps.tile([D + 1, 512], F32, tag="av", name=f"av0_{ch}_{pr}")
                        av1 = avps.tile([D + 1, 512], F32, tag="av", name=f"av1_{ch}_{pr}")

                        def emit_av(st, ex):
                            vs = 128 * (st - 2 - 4 * ch) if st >= 2 + 4 * ch else 0
                            nc.tensor.matmul(av0[:, vs:512], v_s[:, st, j0], ex[:, vs:512],
                                             start=(st == 0), stop=(st == n_st - 1))
                            nc.tensor.matmul(av1[:, vs:512], v_s[:, st, j1], ex[:, 512 + vs:1024],
                                             start=(st == 0), stop=(st == n_st - 1))

                        pend = []
                        for st in range(n_st):
                            diag = st >= 2 + 4 * ch
                            if st < 2:
                                kf0 = memT_s[0:64, rt, 128 * st:128 * st + 128]
                                kf1 = memT_s[64:128, rt, 128 * st:128 * st + 128]
                            else:
                                kf0 = qkT[0:64, 4 + rt, 128 * (st - 2):128 * (st - 2) + 128]
                                kf1 = qkT[64:128, 4 + rt, 128 * (st - 2):128 * (st - 2) + 128]
                            sp = st - 2 - 4 * ch if diag else 0
                            vs = 128 * sp
                            sc = scps.tile([P, 1024], F32, tag="sc", name=f"sc_{ch}_{pr}_{st}")
                            nc.tensor.matmul(sc[:, vs:512], kf0, qT0[:, vs:512],
                                             start=True, stop=not diag)
                            nc.tensor.matmul(sc[:, 512 + vs:1024], kf1, qT1[:, vs:512],
                                             start=True, stop=not diag)
                            if diag:
                                nc.tensor.matmul(sc[:, vs:512], ident_s[:], masks_s[:, sp, vs:512],
                                                 start=False, stop=True)
                                nc.tensor.matmul(sc[:, 512 + vs:1024], ident_s[:],
                                                 masks_s[:, sp, vs:512], start=False, stop=True)
                            ex = expp.tile([P, 1024], F32R, tag="ex", name=f"ex_{ch}_{pr}_{st}")
                            if vs == 0:
                                nc.scalar.activation(ex[:], sc[:], EXP, scale=SCALE)
                            else:
                                nc.scalar.activation(ex[:, vs:512], sc[:, vs:512], EXP, scale=SCALE)
                                nc.scalar.activation(ex[:, 512 + vs:1024], sc[:, 512 + vs:1024],
                                                     EXP, scale=SCALE)
                            pend.append((st, ex))
                            if len(pend) > LAG:
                                emit_av(*pend.pop(0))
                        for st_ex in pend:
                            emit_av(*st_ex)

                        stk = stkp.tile([P, 512], F32R, tag="stk", name=f"stk_{ch}_{pr}")
                        stg0 = stgp.tile([D + 1, 512], F32R, tag="stg", name=f"stg0_{ch}_{pr}")
                        stg1 = stgp.tile([D + 1, 512], F32R, tag="stg", name=f"stg1_{ch}_{pr}")
                        nc.vector.tensor_copy(stg0[:], av0[:])
                        nc.vector.tensor_copy(stg1[:], av1[:])
                        # denominator rows live at partition 64; partition_broadcast
                        # reads absolute partition 0, so DMA-shift them first.
                        denr = bcsp.tile([1, 2, 512], F32R, tag="denr", name=f"dn_{ch}_{pr}")
                        nc.sync.dma_start(denr[:, 0], stg0[D:D + 1, :])
                        nc.sync.dma_start(denr[:, 1], stg1[D:D + 1, :])
                        bcs = bcsp.tile([D, 2, 512], F32R, tag="bcs", name=f"bcs_{ch}_{pr}")
                        nc.gpsimd.partition_broadcast(bcs[:], denr[:])
                        with nc.allow_low_precision(reason="softmax denom reciprocal"):
                            nc.vector.reciprocal(bcs[:], bcs[:])
                        nc.vector.tensor_tensor(stk[0:D, :], stg0[0:D, :], bcs[:, 0],
                                                mybir.AluOpType.mult)
                        nc.vector.tensor_tensor(stg1[0:D, :], stg1[0:D, :], bcs[:, 1],
                                                mybir.AluOpType.mult)
                        # partition-shift the odd head into rows 64..127
                        nc.sync.dma_start(stk[D:2 * D, :], stg1[0:D, :])
                        stks.append(stk)

                    if parts == "noproj":
                        for pr in range(4):
                            ot = outp.tile([P, 512], F32, tag="ot", name=f"od_{ch}_{pr}")
                            nc.vector.tensor_copy(ot[:], stks[pr][:].bitcast(F32))
                            nc.sync.dma_start(out_d[512 * ch + 128 * pr:512 * ch + 128 * pr + 128,
                                                    0:512], ot[:])
                        continue
                    # projection for this chunk (K=128 over 4 pair-stacked tiles)
                    for mt in range(4):
                        for n2 in range(2):
                            pp = scps.tile([P, 1024], F32, tag="sc", name=f"pj_{ch}_{mt}_{n2}")
                            for pr in range(4):
                                nc.tensor.matmul(pp[:, 0:512], stks[pr][:, 128 * mt:128 * mt + 128],
                                                 wp2_s[:, pr, 512 * n2:512 * n2 + 512],
                                                 start=(pr == 0), stop=(pr == 3))
                            ot = outp.tile([P, 512], F32, tag="ot", name=f"ot_{ch}_{mt}_{n2}")
                            nc.vector.tensor_copy(ot[:], pp[:, 0:512])
                            oeng = nc.sync if (mt + n2) % 2 == 0 else nc.gpsimd
                            oeng.dma_start(
                                out_d[512 * ch + 128 * mt:512 * ch + 128 * mt + 128,
                                      512 * n2:512 * n2 + 512], ot[:])

        if parts == "p1":
            with tc.tile_pool(name="dumo", bufs=2) as dumo:
                for mt in range(8):
                    dt_ = dumo.tile([P, T], F32, tag="d")
                    nc.vector.tensor_copy(dt_[:], qkT[:, mt].bitcast(F32))
                    nc.sync.dma_start(out_d[128 * mt:128 * mt + 128, 0:1024], dt_[:, 0:1024])
                dv = dumo.tile([P, 1024], F32, tag="dv")
                nc.vector.tensor_copy(dv[:], v_s[:, 0:2, :, 0:D].bitcast(F32))
                nc.sync.dma_start(out_d[1024:1152, :], dv[:])


def _host_inputs(x, ext_mem, W_attn, b_attn, W_proj, b_proj):
    """Per-core input maps (host-side sharding/layout prep, no FLOPs)."""
    masks = np.zeros((4, P, 512), dtype=np.float32)
    pp_ = np.arange(P)[:, None]
    tt_ = np.arange(512)[None, :]
    for sp in range(4):
        masks[sp] = np.where(pp_ + 128 * sp <= tt_, 0.0, NEG)
    ident = np.eye(P, dtype=np.float32)
    onesrow = np.ones((1, 512), dtype=np.float32)
    ones2 = np.ones((D + 1, D), dtype=np.float32)
    onescol = np.ones((P, NST * HL), dtype=np.float32)

    in_maps = []
    for c in range(8):
        b, g = c // 2, c % 2
        qs = slice(512 * g, 512 * g + 512)
        ks = slice(1024 + 512 * g, 1024 + 512 * g + 512)
        vs = slice(2048 + 512 * g, 2048 + 512 * g + 512)
        mem = np.ascontiguousarray(ext_mem[b][:, 512 * g:512 * g + 512])
        in_maps.append({
            "xT": np.ascontiguousarray(x[b].T),
            "memT": np.ascontiguousarray(mem.T),
            "memn": mem,
            "wqk": np.ascontiguousarray(np.concatenate([W_attn[:, qs], W_attn[:, ks]], axis=1)),
            "wv": np.ascontiguousarray(W_attn[:, vs]),
            "wp": np.ascontiguousarray(W_proj[512 * g:512 * g + 512, :]),
            "bqk": np.concatenate([b_attn[qs], b_attn[ks]])[None, :].astype(np.float32),
            "bv": b_attn[vs][None, :].astype(np.float32),
            "masks": masks, "ident": ident, "onesrow": onesrow,
            "ones2": ones2, "onescol": onescol,
        })
    return in_maps


def kernel(x, ext_mem, W_attn, b_attn, W_proj, b_proj):
    x = np.asarray(x, dtype=np.float32)
    ext_mem = np.asarray(ext_mem, dtype=np.float32)
    W_attn = np.asarray(W_attn, dtype=np.float32)
    b_attn = np.asarray(b_attn, dtype=np.float32)
    W_proj = np.asarray(W_proj, dtype=np.float32)
    b_proj = np.asarray(b_proj, dtype=np.float32)

    if "nc" not in _CACHE:
        _CACHE["nc"] = _build()
    nc = _CACHE["nc"]

    in_maps = _host_inputs(x, ext_mem, W_attn, b_attn, W_proj, b_proj)
    res = run_bass_kernel_spmd(nc, in_maps, list(range(8)))

    out = np.empty((B, T, C), dtype=np.float32)
    for b in range(B):
        out[b] = res.results[2 * b]["out"] + res.results[2 * b + 1]["out"] + b_proj
    return out
